# revision 9
# baseline (speedup 1.0000x reference)
"""Trainium2 Bass kernel for nn_DBLoss_11605001634022.

DBLoss = Ls + Lb + 10*Lt over four (16,640,640) f32 maps, where Ls/Lb are
"balanced" BCE-with-logits losses with hard-negative mining (keep the top
n_negative = min(n_neg_avail, 3*n_pos) negative losses) and
Lt = mean|thresh - target_thresh|.

For these inputs the targets are ~uniform, so n_neg_avail <= 3*n_pos by a
huge margin and the top-k keeps ALL negatives; each balanced BCE collapses
to a plain mean of the elementwise BCE losses. With
bce(x, t) = softplus(x) - x*t, the whole loss is one streaming reduction:

  loss = [ S(sp(p)) - S(p*tp) + S(sp(50*a)) - 2500*S(a*b) + 10*S(|c|) ] / N
  a = p - t,  b = tp - tt,  c = t - tt,   S = sum over all elements

The kernel verifies the collapse condition on the host (cheap) and falls
back to an exact numpy implementation if it ever fails.

The HW has no softplus ACT table, so softplus is decomposed as
  S(sp(x)) = (S(x) + S(|x|))/2 + S(ln(1 + exp(-|x|)))     [relu identity]
with exp/ln/abs all in the single `natural_log_exp_and_others` ACT table
set (one table load, no switches).

Sharded batch-parallel: 2 images/core across 8 cores; each core streams
its 13.1 MB once, in 4 double-buffered [128,1600] chunks. Per-chunk engine
split (raw Bass, manual semaphores — the Tile layer's multi-wait sync is
rejected by this walrus):
  DVE : a=p-t, b=tp-tt, fused (p*-1)*tp and (a*-2500)*b multiply+row-sum
        (scalar_tensor_tensor w/ accum), nap=(p*-1) min p = -|p| w/ free
        row-sum of -|p|.
  ACT : |a| w/ row-sum, exp(-50|a|), ln(1+u) w/ row-sum, exp(-|p|),
        ln(1+u) w/ row-sum, 10|c| w/ row-sum.
  POOL: c = t - tt.
  PE  : ones-matmuls accumulate column sums of p and a into PSUM
        (for the relu identity's S(x) terms).
Row-sums land in independent columns of a [128,28] stats tile; PSUM column
sums are copied to SBUF at the end. Host applies per-column coefficients
and the final division in float64.
"""

import numpy as np

N_CORES = 8
SHAPE = (16, 640, 640)
NTOT = SHAPE[0] * SHAPE[1] * SHAPE[2]
PER_CORE = NTOT // N_CORES  # 819200
P = 128
FDIM = PER_CORE // P  # 6400
NCHUNK = 4
F = FDIM // NCHUNK  # 1600
NCOL = 7  # stats columns per chunk
R = 50.0
ALPHA = 1.0
BETA = 10.0
K = 3

# per-column coefficients (see kernel body):
# 0: S(-p*tp)      -> +1
# 1: S(-2500*a*b)  -> +1
# 2: S(-|p|)       -> -0.5   (relu identity: +S(|p|)/2)
# 3: S(|a|)        -> +25    (S(|50a|)/2)
# 4: S(ln1p(e^-|p|))   -> +1
# 5: S(ln1p(e^-50|a|)) -> +1
# 6: 10*S(|c|)     -> +1
_COEFF = np.array([1.0, 1.0, -0.5, -0.5 * -1.0 * R, 1.0, 1.0, 1.0])
# note: col3 holds S(+|a|), so its coefficient is +R/2 = +25
_COEFF[3] = R / 2.0

_CACHE = {}


def _get_concourse():
    try:
        import concourse.bass  # noqa: F401
    except ImportError:
        import sys

        sys.path.insert(0, "/opt/trn_rl_repo")
    import concourse.bass as bass
    import concourse.mybir as mybir
    from concourse import bass_utils

    return bass, mybir, bass_utils


def _build(nloop=1):
    """Build the bass program. nloop > 1 unrolls the whole pipeline nloop
    times inside one NEFF (same result; used for dispatch-free timing)."""
    if nloop in _CACHE:
        return _CACHE[nloop]
    import contextlib

    bass, mybir, bass_utils = _get_concourse()
    f32 = mybir.dt.float32
    Alu = mybir.AluOpType
    Act = mybir.ActivationFunctionType

    nc = bass.Bass()
    dp = nc.dram_tensor("p", [P, FDIM], f32, kind="ExternalInput")
    dt_ = nc.dram_tensor("t", [P, FDIM], f32, kind="ExternalInput")
    dtp = nc.dram_tensor("tp", [P, FDIM], f32, kind="ExternalInput")
    dtt = nc.dram_tensor("tt", [P, FDIM], f32, kind="ExternalInput")
    dout = nc.dram_tensor("acc_out", [P, NCOL * NCHUNK], f32, kind="ExternalOutput")
    dout2 = nc.dram_tensor("colsum_out", [1, 2 * F], f32, kind="ExternalOutput")

    NB = 2  # buffers
    # PSUM bank slices per chunk column range
    KSL = [(0, 512), (512, 1024), (1024, 1536), (1536, 1600)]

    ctx = contextlib.ExitStack()
    with ctx:
        sb = lambda name, shape: ctx.enter_context(
            nc.sbuf_tensor(name, shape, f32)
        )
        ps = lambda name, shape: ctx.enter_context(
            nc.psum_tensor(name, shape, f32)
        )
        tP = [sb(f"tP{i}", [P, F]) for i in range(NB)]
        tT = [sb(f"tT{i}", [P, F]) for i in range(NB)]
        tTP = [sb(f"tTP{i}", [P, F]) for i in range(NB)]
        tTT = [sb(f"tTT{i}", [P, F]) for i in range(NB)]
        tA = [sb(f"tA{i}", [P, F]) for i in range(NB)]
        tB = [sb(f"tB{i}", [P, F]) for i in range(NB)]
        tC = [sb(f"tC{i}", [P, F]) for i in range(NB)]
        tNP = [sb(f"tNP{i}", [P, F]) for i in range(NB)]
        tAA = [sb(f"tAA{i}", [P, F]) for i in range(NB)]
        tE = sb("tE", [P, F])  # ACT scratch
        tF = sb("tF", [P, F])  # ACT scratch 2
        trash = sb("trash", [P, F])  # DVE scalar_tensor_tensor dump
        # separate per-engine stats tiles: concurrent accum_out writes from
        # different engines never share an SBUF write granule
        acc_d = sb("acc_d", [P, 3 * NCHUNK])  # DVE: stt1, stt2, nap
        acc_a = sb("acc_a", [P, 4 * NCHUNK])  # ACT: abs_a, ln_p, ln_a, abs_c
        csum = sb("csum", [1, 2 * F])  # PSUM column sums staged for DMA-out
        pp = [ps(f"pp{i}", [1, 512]) for i in range(4)]  # sum(p) col partials
        pa = [ps(f"pa{i}", [1, 512]) for i in range(4)]  # sum(a) col partials

        ones = nc.const_aps.tensor(1.0, (P, 1), f32)

        dma_sem = ctx.enter_context(nc.semaphore())
        dve_sem = ctx.enter_context(nc.semaphore())
        act_sem = ctx.enter_context(nc.semaphore())
        pool_sem = ctx.enter_context(nc.semaphore())
        pe_sem = ctx.enter_context(nc.semaphore())
        block = ctx.enter_context(nc.Block())

        def dcol(j, k):
            return acc_d[:, 3 * j + k : 3 * j + k + 1]

        def acol(j, k):
            return acc_a[:, 4 * j + k : 4 * j + k + 1]

        T = nloop * NCHUNK  # total chunk iterations

        @block.sync
        def _(sync):
            for jj in range(T):
                j = jj % NCHUNK
                bi = jj % NB
                sl = slice(j * F, (j + 1) * F)
                if jj >= NB:
                    # input buffers of chunk jj-2 must be fully consumed
                    sync.wait_ge(dve_sem, 5 * (jj - 1))
                    sync.wait_ge(pool_sem, jj - 1)
                    sync.wait_ge(pe_sem, 8 * (jj - 1))
                sync.dma_start(out=tP[bi][:], in_=dp[:, sl]).then_inc(dma_sem, 16)
                sync.dma_start(out=tT[bi][:], in_=dt_[:, sl]).then_inc(dma_sem, 16)
                sync.dma_start(out=tTP[bi][:], in_=dtp[:, sl]).then_inc(dma_sem, 16)
                sync.dma_start(out=tTT[bi][:], in_=dtt[:, sl]).then_inc(dma_sem, 16)
            sync.wait_ge(dve_sem, 5 * T + 8)  # incl. PSUM->SBUF copies
            sync.wait_ge(act_sem, 6 * T)
            sync.wait_ge(pool_sem, T)
            sync.dma_start(
                out=dout[:, : 3 * NCHUNK], in_=acc_d[:]
            ).then_inc(dma_sem, 16)
            sync.dma_start(
                out=dout[:, 3 * NCHUNK :], in_=acc_a[:]
            ).then_inc(dma_sem, 16)
            sync.dma_start(out=dout2[:], in_=csum[:]).then_inc(dma_sem, 16)
            sync.wait_ge(dma_sem, 64 * T + 48)

        @block.vector
        def _(vector):
            for jj in range(T):
                j = jj % NCHUNK
                bi = jj % NB
                vector.wait_ge(dma_sem, 64 * (jj + 1))
                if jj >= NB:
                    # tA[bi] readers from chunk jj-2: ACT abs_a (act op1),
                    # PE mm_a (pe 8(jj-1)); tNP[bi] reader: ACT e1 (act op4)
                    vector.wait_ge(act_sem, 6 * (jj - 2) + 1)
                    vector.wait_ge(pe_sem, 8 * (jj - 1))
                nc.vector.tensor_sub(
                    out=tA[bi][:], in0=tP[bi][:], in1=tT[bi][:]
                ).then_inc(dve_sem, 1)
                nc.vector.tensor_sub(
                    out=tB[bi][:], in0=tTP[bi][:], in1=tTT[bi][:]
                ).then_inc(dve_sem, 1)
                nc.vector.scalar_tensor_tensor(
                    out=trash[:], in0=tP[bi][:], scalar=-1.0, in1=tTP[bi][:],
                    op0=Alu.mult, op1=Alu.mult, accum_out=dcol(j, 0),
                ).then_inc(dve_sem, 1)
                nc.vector.scalar_tensor_tensor(
                    out=trash[:], in0=tA[bi][:], scalar=-2500.0, in1=tB[bi][:],
                    op0=Alu.mult, op1=Alu.mult, accum_out=dcol(j, 1),
                ).then_inc(dve_sem, 1)
                if jj >= NB:
                    vector.wait_ge(act_sem, 6 * (jj - 2) + 4)  # e1 read tNP[bi]
                nc.vector.scalar_tensor_tensor(
                    out=tNP[bi][:], in0=tP[bi][:], scalar=-1.0, in1=tP[bi][:],
                    op0=Alu.mult, op1=Alu.min, accum_out=dcol(j, 2),
                ).then_inc(dve_sem, 1)
            # PSUM -> SBUF staging
            vector.wait_ge(pe_sem, 8 * T)
            for k, (lo, hi) in enumerate(KSL):
                w = hi - lo
                nc.vector.tensor_copy(
                    out=csum[0:1, lo:hi], in_=pp[k][0:1, 0:w]
                ).then_inc(dve_sem, 1)
            for k, (lo, hi) in enumerate(KSL):
                w = hi - lo
                nc.vector.tensor_copy(
                    out=csum[0:1, F + lo : F + hi], in_=pa[k][0:1, 0:w]
                ).then_inc(dve_sem, 1)

        @block.scalar
        def _(scalar):
            for jj in range(T):
                j = jj % NCHUNK
                bi = jj % NB
                scalar.wait_ge(dve_sem, 5 * jj + 1)  # tA ready
                nc.scalar.activation(
                    tAA[bi][:], tA[bi][:], Act.Abs, accum_out=acol(j, 0)
                ).then_inc(act_sem, 1)
                nc.scalar.activation(
                    tE[:], tAA[bi][:], Act.Exp, scale=-R
                ).then_inc(act_sem, 1)
                nc.scalar.activation(
                    tF[:], tE[:], Act.Ln, bias=1.0, accum_out=acol(j, 2)
                ).then_inc(act_sem, 1)
                scalar.wait_ge(dve_sem, 5 * jj + 5)  # tNP ready
                nc.scalar.activation(tE[:], tNP[bi][:], Act.Exp).then_inc(act_sem, 1)
                nc.scalar.activation(
                    tF[:], tE[:], Act.Ln, bias=1.0, accum_out=acol(j, 1)
                ).then_inc(act_sem, 1)
                scalar.wait_ge(pool_sem, jj + 1)  # tC ready
                nc.scalar.activation(
                    tE[:], tC[bi][:], Act.Abs, scale=BETA, accum_out=acol(j, 3)
                ).then_inc(act_sem, 1)

        @block.gpsimd
        def _(gpsimd):
            for jj in range(T):
                bi = jj % NB
                gpsimd.wait_ge(dma_sem, 64 * (jj + 1))
                if jj >= NB:
                    gpsimd.wait_ge(act_sem, 6 * (jj - 2) + 6)  # abs_c read tC[bi]
                nc.gpsimd.tensor_sub(
                    out=tC[bi][:], in0=tT[bi][:], in1=tTT[bi][:]
                ).then_inc(pool_sem, 1)

        @block.tensor
        def _(tensor):
            for jj in range(T):
                j = jj % NCHUNK
                bi = jj % NB
                tensor.wait_ge(dma_sem, 64 * (jj + 1))
                for k, (lo, hi) in enumerate(KSL):
                    w = hi - lo
                    nc.tensor.matmul(
                        pp[k][0:1, 0:w],
                        ones,
                        tP[bi][:, lo:hi],
                        start=(j == 0),
                        stop=(j == NCHUNK - 1),
                    ).then_inc(pe_sem, 1)
                tensor.wait_ge(dve_sem, 5 * jj + 1)  # tA ready
                for k, (lo, hi) in enumerate(KSL):
                    w = hi - lo
                    nc.tensor.matmul(
                        pa[k][0:1, 0:w],
                        ones,
                        tA[bi][:, lo:hi],
                        start=(j == 0),
                        stop=(j == NCHUNK - 1),
                    ).then_inc(pe_sem, 1)

    _CACHE[nloop] = (nc, bass_utils)
    return _CACHE[nloop]


def _run_device(shards, **kwargs):
    """shards: dict name -> list of 8 [P, FDIM] f32 arrays."""
    nc, bass_utils = _build()
    in_maps = [
        {name: shards[name][c] for name in ("p", "t", "tp", "tt")}
        for c in range(N_CORES)
    ]
    return bass_utils.run_bass_kernel_spmd(
        nc, in_maps, core_ids=list(range(N_CORES)), **kwargs
    )


def _shard(arr):
    flat = np.ascontiguousarray(arr, dtype=np.float32).reshape(-1)
    return [
        flat[c * PER_CORE : (c + 1) * PER_CORE].reshape(P, FDIM)
        for c in range(N_CORES)
    ]


def _reduce_host(results):
    # acc_out columns: [0:12] DVE (chunk-major: stt1, stt2, nap),
    # [12:28] ACT (chunk-major: abs_a, ln_p, ln_a, abs_c)
    cd = np.array([1.0, 1.0, -0.5])
    ca = np.array([R / 2.0, 1.0, 1.0, 1.0])
    total = 0.0
    for c in range(N_CORES):
        out = results[c]["acc_out"].astype(np.float64)
        dve = out[:, : 3 * NCHUNK].reshape(P, NCHUNK, 3)
        act = out[:, 3 * NCHUNK :].reshape(P, NCHUNK, 4)
        total += float((dve.sum(axis=(0, 1)) * cd).sum())
        total += float((act.sum(axis=(0, 1)) * ca).sum())
        cs = results[c]["colsum_out"].astype(np.float64).reshape(2 * F)
        total += 0.5 * cs[:F].sum()  # S(p)/2
        total += (R / 2.0) * cs[F:].sum()  # 25*S(a)
    return np.float32(total / NTOT)


def _numpy_fallback(p, t, tp, tt):
    """Exact reference semantics in float32 numpy (only used if the top-k
    collapse precondition ever fails)."""

    def bce(x, tgt):
        return (
            np.maximum(x, 0.0) - x * tgt + np.log1p(np.exp(-np.abs(x)))
        ).astype(np.float32)

    def balanced(x, tgt):
        losses = bce(x, tgt).ravel()
        mask = tgt.ravel() > 0.5
        n_pos = int(mask.sum())
        n_neg_avail = mask.size - n_pos
        n_negative = min(n_neg_avail, K * n_pos)
        pos_sum = np.float32(losses[mask].sum())
        neg_sorted = np.sort(losses[~mask])[::-1]
        neg_sum = np.float32(neg_sorted[:n_negative].sum())
        return (pos_sum + neg_sum) / np.float32(n_pos + n_negative)

    bin_map = (R * (p - t)).astype(np.float32)
    target_bin = (R * (tp - tt)).astype(np.float32)
    ls = balanced(p, tp)
    lb = balanced(bin_map, target_bin)
    lt = np.abs(t - tt).mean(dtype=np.float32)
    return np.float32(ls + ALPHA * lb + BETA * lt)


def kernel(
    proba_map, thresh_map, target_proba_map, target_thresh_map
) -> np.ndarray:
    p = np.asarray(proba_map, dtype=np.float32)
    t = np.asarray(thresh_map, dtype=np.float32)
    tp = np.asarray(target_proba_map, dtype=np.float32)
    tt = np.asarray(target_thresh_map, dtype=np.float32)

    # The device kernel assumes the hard-negative top-k keeps every negative
    # (n_neg_avail <= K*n_pos for both BCE terms). Cheap host check; exact
    # fallback otherwise.
    npos1 = int(np.count_nonzero(tp > 0.5))
    d = (R * (tp - tt)).astype(np.float32)
    npos2 = int(np.count_nonzero(d > 0.5))
    if (tp.size - npos1) > K * npos1 or (d.size - npos2) > K * npos2:
        return _numpy_fallback(p, t, tp, tt)

    shards = {"p": _shard(p), "t": _shard(t), "tp": _shard(tp), "tt": _shard(tt)}
    res = _run_device(shards)
    return _reduce_host(res.results)
